# revision 9
# baseline (speedup 1.0000x reference)
"""Trainium2 Bass kernel for nn_CELoss_51634096832929.

Label-smoothed, ignore-index(0) cross-entropy with 'mean over selected
weights' reduction, over input [8, 14, 512, 512] f32 / target [8, 512, 512].

Math (per pixel, C=14, eps=0.1, a = eps/(C-1)):
    lse  = log(sum_c exp(x_c))
    loss = c1*sum_c x_c + c2*lse + c3*x_0 + c4*x_t + c5*is0*x_0 - c5*is0*lse
      c1 = -a, c2 = 0.9 + 11a, c3 = 2a, c4 = -(0.9 - a), c5 = 1.8 - 2a
    s_weight_sum = k1 + k2*is0   (k1 = 0.9 + 12a, k2 = 0.1 - k1)
    out = sum_{loss>0} loss / sum_{loss>0} s_weight_sum

Sharding: pure data-parallel, batch n -> core n (8 batches, 8 cores).
Each core reduces its batch to per-partition partial sums; the final
combine (tiny) happens on the host.

Per-core dataflow (pixel-major layout [128 partitions, 2048 free]):
  - stream 14 channel planes: DMA -> ACT exp -> DVE (t==c)*x_c
  - PE identity-matmuls accumulate, per 512-col PSUM block:
      psumA = sum_c exp(x_c)
      psumB = c1*sum_c x_c (+c3 on c=0) + c4*sum_c q_c (+c5 on c=0)
  - tail per block: ACT ln -> PE adds c2*lse and -c5*is0*lse into psumB
    (psumB becomes per-pixel loss), ACT relu(+accum) -> loss_sel partials,
    DVE is_gt(+accum) -> npos partials, DVE (t==0)*pos(+accum) -> npos0.
"""

import numpy as np
from contextlib import ExitStack

import concourse.bacc as bacc
import concourse.bass as bass
import concourse.tile as tile
from concourse import mybir
from concourse.bass_utils import run_bass_kernel_spmd

AF = mybir.ActivationFunctionType
OP = mybir.AluOpType
F32 = mybir.dt.float32
I32 = mybir.dt.int32

N_CORES = 8
C = 14
H = 512
W = 512
PIX = H * W          # 262144 pixels per batch
P = 128              # SBUF partitions
FW = PIX // P        # 2048 free-dim columns per partition
NBLK = 4             # PSUM blocks
BLK = FW // NBLK     # 512 columns per block (one PSUM bank)

EPS = 0.1
A = EPS / (C - 1)
C1 = -A
C2 = 0.9 + 11.0 * A
C3 = 2.0 * A
C4 = -(0.9 - A)
C5 = 1.8 - 2.0 * A
K1 = 0.9 + 12.0 * A
K2 = 0.1 - K1

_CACHE = {}


def _build():
    nc = bacc.Bacc("TRN2", target_bir_lowering=False)

    x = nc.declare_dram_parameter("x", [C, H, W], F32, isOutput=False)
    tg = nc.declare_dram_parameter("tg", [H, W], I32, isOutput=False)
    acc = nc.declare_dram_parameter("acc", [3, P, NBLK], F32, isOutput=True)

    # Identity-matrix weight variants for the PE accumulation matmuls.
    eye = np.eye(P, dtype=np.float32)
    w_np = np.stack(
        [
            eye,                    # 0: sumexp accumulate
            np.float32(C1) * eye,   # 1: x_c  (c >= 1)
            np.float32(C1 + C3) * eye,  # 2: x_0
            np.float32(C4) * eye,   # 3: q_c  (c >= 1)
            np.float32(C4 + C5) * eye,  # 4: q_0
            np.float32(C2) * eye,   # 5: lse
            np.float32(-C5) * eye,  # 6: is0*lse
        ]
    )
    wd = nc.inline_tensor(w_np, name="wvars")

    xv = x[:].rearrange("c h w -> c (h w)").rearrange("c (p f) -> c p f", p=P)
    tv = tg[:].rearrange("h w -> (h w)").rearrange("(p f) -> p f", p=P)
    accv = acc[:]

    with tile.TileContext(nc) as tc, ExitStack() as ctx:
        consts = ctx.enter_context(tc.tile_pool(name="consts", bufs=1))
        xpool = ctx.enter_context(tc.tile_pool(name="xpool", bufs=4))
        epool = ctx.enter_context(tc.tile_pool(name="epool", bufs=4))
        qpool = ctx.enter_context(tc.tile_pool(name="qpool", bufs=4))
        spool = ctx.enter_context(tc.tile_pool(name="spool", bufs=4))
        psa = ctx.enter_context(tc.tile_pool(name="psa", bufs=1, space="PSUM"))
        psb = ctx.enter_context(tc.tile_pool(name="psb", bufs=1, space="PSUM"))

        wsb = consts.tile([P, 7, P], F32)
        nc.sync.dma_start(out=wsb, in_=wd[:].rearrange("i k m -> k i m"))
        wI = wsb[:, 0, :]
        wX = wsb[:, 1, :]
        wX0 = wsb[:, 2, :]
        wQ = wsb[:, 3, :]
        wQ0 = wsb[:, 4, :]
        wL = wsb[:, 5, :]
        wU = wsb[:, 6, :]

        t32 = consts.tile([P, FW], I32)
        nc.sync.dma_start(out=t32, in_=tv)
        tf = consts.tile([P, FW], F32)
        nc.vector.tensor_copy(out=tf, in_=t32)
        # Joiner: absorbs the DVE self-wait for tf so later DVE ops that also
        # depend on a fresh DMA carry only one sync wait (HW struct limit).
        tfj = consts.tile([P, 1], F32)
        nc.vector.tensor_copy(out=tfj, in_=tf[:, 0:1])

        accL = consts.tile([P, NBLK], F32)
        accP = consts.tile([P, NBLK], F32)
        accQ = consts.tile([P, NBLK], F32)

        pa = [psa.tile([P, BLK], F32, name=f"pa{j}") for j in range(NBLK)]
        pb = [psb.tile([P, BLK], F32, name=f"pb{j}") for j in range(NBLK)]

        # Tiny warm-up matmuls so PE observes the weights-DMA semaphore once;
        # real matmuls then carry at most one sync wait (walrus's LDW struct
        # only has room for a single wait command).
        for i in range(7):
            nc.tensor.matmul(
                pa[0][:, 0:1], wsb[:, i, :], wsb[:, 0, 0:1], start=True, stop=True
            )

        for c in range(C):
            xc = xpool.tile([P, FW], F32, name="xc")
            nc.sync.dma_start(out=xc, in_=xv[c])
            ec = epool.tile([P, FW], F32, name="ec")
            nc.scalar.activation(out=ec, in_=xc, func=AF.Exp)
            qc = qpool.tile([P, FW], F32, name="qc")
            nc.vector.scalar_tensor_tensor(
                out=qc, in0=tf, scalar=float(c), in1=xc, op0=OP.is_equal, op1=OP.mult
            )
            for j in range(NBLK):
                sl = slice(j * BLK, (j + 1) * BLK)
                nc.tensor.matmul(
                    pa[j], wI, ec[:, sl], start=(c == 0), stop=(c == C - 1)
                )
                nc.tensor.matmul(
                    pb[j], wX0 if c == 0 else wX, xc[:, sl],
                    start=(c == 0), stop=False,
                )
                nc.tensor.matmul(
                    pb[j], wQ0 if c == 0 else wQ, qc[:, sl],
                    start=False, stop=False,
                )

        for j in range(NBLK):
            sl = slice(j * BLK, (j + 1) * BLK)
            lse = spool.tile([P, BLK], F32, name="lse")
            nc.scalar.activation(out=lse, in_=pa[j], func=AF.Ln)
            nc.tensor.matmul(pb[j], wL, lse, start=False, stop=False)
            u = spool.tile([P, BLK], F32, name="u")
            nc.vector.scalar_tensor_tensor(
                out=u, in0=tf[:, sl], scalar=0.0, in1=lse,
                op0=OP.is_equal, op1=OP.mult,
            )
            nc.tensor.matmul(pb[j], wU, u, start=False, stop=True)

            lr = spool.tile([P, BLK], F32, name="lr")
            nc.scalar.activation(
                out=lr, in_=pb[j], func=AF.Relu, accum_out=accL[:, j : j + 1]
            )
            pos = spool.tile([P, BLK], F32, name="pos")
            nc.vector.tensor_scalar(
                out=pos, in0=lr, scalar1=0.0, scalar2=0.0, op0=OP.is_gt,
                op1=OP.add, accum_out=accP[:, j : j + 1],
            )
            pi = spool.tile([P, BLK], F32, name="pi")
            nc.vector.scalar_tensor_tensor(
                out=pi, in0=tf[:, sl], scalar=0.0, in1=pos,
                op0=OP.is_equal, op1=OP.mult,
                accum_out=accQ[:, j : j + 1],
            )

        nc.sync.dma_start(out=accv[0], in_=accL)
        nc.sync.dma_start(out=accv[1], in_=accP)
        nc.sync.dma_start(out=accv[2], in_=accQ)

    nc.compile()
    return nc


def get_nc():
    if "nc" not in _CACHE:
        _CACHE["nc"] = _build()
    return _CACHE["nc"]


def run_cores(input, target, **kw):
    """Run the SPMD kernel; returns (BassKernelResults, per-core acc list)."""
    x = np.asarray(input)
    if x.dtype != np.float32:
        x = x.astype(np.float32)
    t = np.asarray(target)
    t32 = t.astype(np.int32) if t.dtype != np.int32 else t

    nc = get_nc()
    in_maps = [
        {"x": np.ascontiguousarray(x[k]), "tg": np.ascontiguousarray(t32[k])}
        for k in range(N_CORES)
    ]
    res = run_bass_kernel_spmd(nc, in_maps, core_ids=list(range(N_CORES)), **kw)
    accs = [res.results[k]["acc"] for k in range(N_CORES)]
    return res, accs


def combine(accs):
    loss_sel = 0.0
    npos = 0.0
    npos0 = 0.0
    for a in accs:
        loss_sel += a[0].sum(dtype=np.float64)
        npos += a[1].sum(dtype=np.float64)
        npos0 += a[2].sum(dtype=np.float64)
    sw_sel = K1 * npos + K2 * npos0
    denom = sw_sel if sw_sel != 0.0 else 1.0
    return np.array(loss_sel / denom, dtype=np.float32)


def kernel(input, target):
    _, accs = run_cores(input, target)
    return combine(accs)


# revision 26
# speedup vs baseline: 2.6999x; 2.6999x over previous
"""Trainium2 Bass kernel for nn_CELoss_51634096832929.

Label-smoothed, ignore-index(0) cross-entropy with 'mean over selected
weights' reduction, over input [8, 14, 512, 512] f32 / target [8, 512, 512].

Math (per pixel, C=14, eps=0.1, a = eps/(C-1)):
    lse  = log(sum_c exp(x_c))
    loss = c1*sum_c x_c + c2*lse + c3*x_0 + c4*x_t + c5*is0*x_0 - c5*is0*lse
      c1 = -a, c2 = 0.9 + 11a, c3 = 2a, c4 = -(0.9 - a), c5 = 1.8 - 2a
    s_weight_sum = k1 + k2*is0   (k1 = 0.9 + 12a, k2 = 0.1 - k1)
    out = sum_{loss>0} loss / sum_{loss>0} s_weight_sum

Sharding: pure data parallel, batch n -> NeuronCore n (8 batches, 8 cores).
Each core reduces its batch to 128x12 per-partition partial sums (loss_sel,
npos, npos*is0); the final all-reduce + divide (tiny) happens on the host.

Per-core dataflow (pixel-major layout, 128 partitions x 2048 columns, split
into a 1536-col phase + a 512-col phase so the final PSUM tail is short):
  - stream the 14 channel planes: DMA chunk -> ACT exp (out bf16)
    -> DVE scalar_tensor_tensor (t==c)*x_c (out bf16)
  - PE identity-matmul accumulation per 512-col PSUM bank (all-bf16 MMs:
    x enters as a strided bf16 view of the fp32 data):
      psumA = sum_c exp(x_c)
      psumB = c1*sum_c x_c (+c3 on c=0) + c4*sum_c q_c (+c5 on c=0)
  - tail per bank: ACT ln -> PE adds c2*lse and -c5*is0*lse into psumB
    (bf16 weight pairs main+residual keep coefficient accuracy), ACT
    relu(+accum) -> loss_sel partials, DVE is_gt(+accum) -> npos partials,
    DVE (t==0)*pos(+accum) -> npos0 partials; one DMA out of [128, 12].

Engine budget per core (~66 us total): DMA ~47 us (15.2 MB @ ~330 GB/s),
PE ~48 us (189 bf16 matmuls), DVE ~53 us, ACT ~52 us, plus ~7 us Tile
preamble and ~9 us exit-barrier postamble.
"""

import numpy as np
from contextlib import ExitStack

import concourse.bacc as bacc
import concourse.bass as bass
import concourse.tile as tile
from concourse import mybir
from concourse.bass_utils import run_bass_kernel_spmd

AF = mybir.ActivationFunctionType
OP = mybir.AluOpType
F32 = mybir.dt.float32
F32R = mybir.dt.float32r
BF16 = mybir.dt.bfloat16
I8 = mybir.dt.int8

N_CORES = 8
C = 14
H = 512
W = 512
PIX = H * W          # 262144 pixels per batch
P = 128              # SBUF partitions
FW = PIX // P        # 2048 free-dim columns per partition
SUB = 512            # columns per PSUM bank
PHASES = [(0, 3), (3, 1)]  # (first sub, n subs): big phase + short last phase

EPS = 0.1
A = EPS / (C - 1)
C1 = -A
C2 = 0.9 + 11.0 * A
C3 = 2.0 * A
C4 = -(0.9 - A)
C5 = 1.8 - 2.0 * A
K1 = 0.9 + 12.0 * A
K2 = 0.1 - K1

_CACHE = {}


def _setup_act_root():
    """Point walrus at an act_info.json whose first exp/ln-capable set is
    natural_log_exp_and_others, so Exp and Ln share one table load."""
    import json
    import os

    if os.environ.get("BASS_ACT_ROOT_JSON_PATH"):
        return
    try:
        _setup_act_root_impl(json, os)
    except Exception:
        os.environ.pop("BASS_ACT_ROOT_JSON_PATH", None)


def _setup_act_root_impl(json, os):
    try:
        import neuronxcc

        src = os.path.join(
            os.path.dirname(neuronxcc.__file__),
            "pwp",
            "pwp_bin_trainium",
            "act_info.json",
        )
    except Exception:
        src = None
    if not src or not os.path.isfile(src):
        return
    srcdir = os.path.dirname(src)
    dst = "/tmp/bass_act_root"
    os.makedirs(dst, exist_ok=True)
    for f in os.listdir(srcdir):
        link = os.path.join(dst, f)
        if not os.path.exists(link):
            try:
                os.symlink(os.path.join(srcdir, f), link)
            except OSError:
                pass
    d = json.load(open(src))
    sets = d.get("act_func_sets", [])
    pref = [s for s in sets if s.get("name") == "natural_log_exp_and_others"]
    rest = [s for s in sets if s.get("name") != "natural_log_exp_and_others"]
    d["act_func_sets"] = pref + rest
    with open(os.path.join(dst, "act_info.json"), "w") as f:
        json.dump(d, f)
    os.environ["BASS_ACT_ROOT_JSON_PATH"] = os.path.join(dst, "act_info.json")


_setup_act_root()


def _build():
    nc = bacc.Bacc("TRN2", target_bir_lowering=False)

    x = nc.declare_dram_parameter("x", [C, H, W], F32, isOutput=False)
    tg = nc.declare_dram_parameter("tg", [H, W], I8, isOutput=False)
    acc = nc.declare_dram_parameter("acc", [P, 12], F32, isOutput=True)

    # Identity-matrix weight variants for the PE accumulation matmuls (bf16).
    # The two big per-pixel coefficients (lse, is0*lse) use residual weight
    # pairs so the effective coefficient keeps ~fp32 accuracy.
    import ml_dtypes

    bf = ml_dtypes.bfloat16

    def b(v):
        return float(np.asarray(v, dtype=bf).astype(np.float32))

    eye = np.eye(P, dtype=np.float32)
    w_np = np.stack(
        [
            eye,                     # 0: sumexp accumulate
            np.float32(C1) * eye,    # 1: x_c  (c >= 1)
            np.float32(C1 + C3) * eye,   # 2: x_0
            np.float32(C4) * eye,    # 3: q_c  (c >= 1)
            np.float32(C4 + C5) * eye,   # 4: q_0
            np.float32(C2) * eye,    # 5: lse (main)
            np.float32(C2 - b(C2)) * eye,    # 6: lse (residual)
            np.float32(-C5) * eye,   # 7: is0*lse (main)
            np.float32(-C5 - b(-C5)) * eye,  # 8: is0*lse (residual)
        ]
    ).astype(bf)
    wd = nc.inline_tensor(w_np, name="wvars")

    xv = x[:].rearrange("c h w -> c (h w)").rearrange("c (p f) -> c p f", p=P)
    tv = tg[:].rearrange("h w -> (h w)").rearrange("(p f) -> p f", p=P)
    accv = acc[:]

    with tile.TileContext(nc) as tc, ExitStack() as ctx:
        consts = ctx.enter_context(tc.tile_pool(name="consts", bufs=1))
        xpool = ctx.enter_context(tc.tile_pool(name="xpool", bufs=8))
        epool = ctx.enter_context(tc.tile_pool(name="epool", bufs=6))
        qpool = ctx.enter_context(tc.tile_pool(name="qpool", bufs=6))
        spool = ctx.enter_context(tc.tile_pool(name="spool", bufs=2))
        psa = ctx.enter_context(tc.tile_pool(name="psa", bufs=2, space="PSUM"))
        psb = ctx.enter_context(tc.tile_pool(name="psb", bufs=2, space="PSUM"))

        wsb = consts.tile([P, 9, P], BF16)
        nc.sync.dma_start(out=wsb, in_=wd[:].rearrange("i k m -> k i m"))
        wI = wsb[:, 0, :]
        wX = wsb[:, 1, :]
        wX0 = wsb[:, 2, :]
        wQ = wsb[:, 3, :]
        wQ0 = wsb[:, 4, :]
        wL1 = wsb[:, 5, :]
        wL2 = wsb[:, 6, :]
        wU1 = wsb[:, 7, :]
        wU2 = wsb[:, 8, :]

        tf = consts.tile([P, FW], I8)
        nc.sync.dma_start(out=tf, in_=tv)
        # Joiner: absorbs the DVE wait on the target DMA so later DVE ops
        # that also depend on a fresh x-chunk DMA carry only one sync wait
        # (the DVE op struct has room for a single wait command).
        tfj = consts.tile([P, 1], F32)
        nc.vector.tensor_copy(out=tfj, in_=tf[:, 0:1])

        acct = consts.tile([P, 12], F32)
        accL = acct[:, 0:4]
        accP = acct[:, 4:8]
        accQ = acct[:, 8:12]

        # Tiny warm-up matmuls so PE observes the weights-DMA semaphore once;
        # real matmuls then carry at most one sync wait (walrus's LDW struct
        # only has room for a single wait command).
        pwarm = psa.tile([P, 8], F32, name="pwarm", tag="pa0")
        for i in range(9):
            nc.tensor.matmul(
                pwarm, wsb[:, i, :], wsb[:, 0, 0:8], start=True, stop=True
            )

        # Column phases: a large leading phase and a short trailing phase so
        # the final (serial) PSUM tail is as short as possible.
        for s0, ns in PHASES:
            sl = slice(s0 * SUB, (s0 + ns) * SUB)
            width = ns * SUB
            pa = [
                psa.tile([P, SUB], F32, name=f"pa{k}", tag=f"pa{k}",
                         bufs=(2 if k == 0 else 1))
                for k in range(ns)
            ]
            pb = [
                psb.tile([P, SUB], F32, name=f"pb{k}", tag=f"pb{k}",
                         bufs=(2 if k == 0 else 1))
                for k in range(ns)
            ]
            for c in range(C):
                xc = xpool.tile([P, width], F32, name="xc")
                nc.sync.dma_start(out=xc, in_=xv[c][:, sl])
                xb = xc.bitcast(BF16)[:, 1::2]
                ec = epool.tile([P, width], BF16, name="ec")
                nc.scalar.activation(out=ec, in_=xc, func=AF.Exp)
                qc = qpool.tile([P, width], BF16, name="qc")
                nc.vector.scalar_tensor_tensor(
                    out=qc, in0=tf[:, sl], scalar=float(c), in1=xc,
                    op0=OP.is_equal, op1=OP.mult,
                )
                for k in range(ns):
                    s2 = slice(k * SUB, (k + 1) * SUB)
                    nc.tensor.matmul(
                        pa[k], wI, ec[:, s2], start=(c == 0), stop=(c == C - 1)
                    )
                    nc.tensor.matmul(
                        pb[k], wX0 if c == 0 else wX, xb[:, s2],
                        start=(c == 0), stop=False,
                    )
                    nc.tensor.matmul(
                        pb[k], wQ0 if c == 0 else wQ, qc[:, s2],
                        start=False, stop=False,
                    )

            for k in range(ns):
                g = s0 + k
                gsl = slice(g * SUB, (g + 1) * SUB)
                lse = spool.tile([P, SUB], BF16, name="lse", bufs=4)
                nc.scalar.activation(out=lse, in_=pa[k], func=AF.Ln)
                nc.tensor.matmul(pb[k], wL1, lse, start=False, stop=False)
                nc.tensor.matmul(pb[k], wL2, lse, start=False, stop=False)
                u = spool.tile([P, SUB], BF16, name="u", bufs=4)
                nc.vector.scalar_tensor_tensor(
                    out=u, in0=tf[:, gsl], scalar=0.0, in1=lse,
                    op0=OP.is_equal, op1=OP.mult,
                )
                nc.tensor.matmul(pb[k], wU1, u, start=False, stop=False)
                nc.tensor.matmul(pb[k], wU2, u, start=False, stop=True)
                lr = spool.tile([P, SUB], F32, name="lr", bufs=4)
                nc.scalar.activation(
                    out=lr, in_=pb[k], func=AF.Relu,
                    accum_out=accL[:, g : g + 1],
                )
                pos = spool.tile([P, SUB], F32, name="pos", bufs=4)
                nc.vector.tensor_scalar(
                    out=pos, in0=lr, scalar1=0.0, scalar2=0.0, op0=OP.is_gt,
                    op1=OP.add, accum_out=accP[:, g : g + 1],
                )
                pi = spool.tile([P, SUB], F32, name="pi", bufs=4)
                nc.vector.scalar_tensor_tensor(
                    out=pi, in0=tf[:, gsl], scalar=0.0, in1=pos,
                    op0=OP.is_equal, op1=OP.mult,
                    accum_out=accQ[:, g : g + 1],
                )

        nc.sync.dma_start(out=accv, in_=acct)

    nc.compile()
    return nc


def get_nc():
    if "nc" not in _CACHE:
        _CACHE["nc"] = _build()
    return _CACHE["nc"]


def run_cores(input, target, **kw):
    """Run the SPMD kernel; returns (BassKernelResults, per-core acc list)."""
    x = np.asarray(input)
    if x.dtype != np.float32:
        x = x.astype(np.float32)
    t = np.asarray(target)
    t8 = t.astype(np.int8)

    nc = get_nc()
    in_maps = [
        {"x": np.ascontiguousarray(x[k]), "tg": np.ascontiguousarray(t8[k])}
        for k in range(N_CORES)
    ]
    res = run_bass_kernel_spmd(nc, in_maps, core_ids=list(range(N_CORES)), **kw)
    accs = [res.results[k]["acc"].reshape(P, 3, 4).transpose(1, 0, 2) for k in range(N_CORES)]
    return res, accs


def combine(accs):
    loss_sel = 0.0
    npos = 0.0
    npos0 = 0.0
    for a in accs:
        loss_sel += a[0].sum(dtype=np.float64)
        npos += a[1].sum(dtype=np.float64)
        npos0 += a[2].sum(dtype=np.float64)
    sw_sel = K1 * npos + K2 * npos0
    denom = sw_sel if sw_sel != 0.0 else 1.0
    return np.array(loss_sel / denom, dtype=np.float32)


def kernel(input, target):
    _, accs = run_cores(input, target)
    return combine(accs)
